# revision 5
# baseline (speedup 1.0000x reference)
"""MultiHeadAttention forward on 8 Trainium2 NeuronCores (Bass/Tile).

Problem: B=2, S=2048, D=1024, H=16 heads (dk=64), fp32, mask all-ones.

Sharding: core c = b*4 + g handles batch b and head group g (4 heads).
Data parallel over B, tensor parallel over heads; w_o row-wise with the
partial-output reduction done host-side (summing 4 fp32 partials).

Device kernel per core (all matmuls in float32r = full-rate fp32):
  1. projections: qhT/khT = (w q)^T layouts [256, 2048] (head dim on
     partitions), vh natural [s, dv] per k-tile, biases fused.
  2. attention per q-chunk of 256: scores k-major [k, q] via row-packed
     K=64 head pairs; exp on ScalarE (PSUM -> SBUF, strided over 4
     half-used banks); PV with stationary [vh | ones] so the softmax
     denominator lands replicated on partitions 64-127 of the ctx bank.
  3. normalize: den -> DMA partition shift -> reciprocal_approx -> TT mul,
     writing the stacked ctx^T tiles used as the output-proj stationary.
  4. output projection -> partial out [2048, 1024] per core.

Host: shards/transposes inputs, runs SPMD over 8 cores, sums group
partials per batch, adds bo.
"""
import math

import numpy as np

B, S, D, H = 2, 2048, 1024, 16
DK = D // H          # 64
HPC = H // 4         # 4 heads per core
NCORES = 8
NT = S // 128        # 16 k-tiles / s-tiles
ND = D // 128        # 8 d-tiles
QC = 256             # q-chunk (f32r moving-operand limit)
NQC = S // QC        # 8
GH = HPC * DK        # 256 output dims per group

_STATE = {}


def _build(loop_r=1):
    """Build the Bass program (shared by all 8 cores; inputs differ)."""
    from contextlib import ExitStack

    import concourse.tile as tile
    from concourse import bacc, mybir

    F32 = mybir.dt.float32
    F32R = mybir.dt.float32r
    EXP = mybir.ActivationFunctionType.Exp

    nc = bacc.Bacc("TRN2", target_bir_lowering=False, debug=False,
                   num_devices=NCORES)

    qT_ext = nc.dram_tensor("qT", [D, S], F32R, kind="ExternalInput").ap()
    kT_ext = nc.dram_tensor("kT", [D, S], F32R, kind="ExternalInput").ap()
    vT_ext = nc.dram_tensor("vT", [D, S], F32R, kind="ExternalInput").ap()
    wqT_ext = nc.dram_tensor("wqT", [D, GH], F32R, kind="ExternalInput").ap()
    wkT_ext = nc.dram_tensor("wkT", [D, GH], F32R, kind="ExternalInput").ap()
    wvT_ext = nc.dram_tensor("wvT", [D, GH], F32R, kind="ExternalInput").ap()
    woT_ext = nc.dram_tensor("woT", [GH, D], F32R, kind="ExternalInput").ap()
    bq_ext = nc.dram_tensor("bq", [GH, 1], F32, kind="ExternalInput").ap()
    bk_ext = nc.dram_tensor("bk", [GH, 1], F32, kind="ExternalInput").ap()
    bv_ext = nc.dram_tensor("bv", [1, GH], F32R, kind="ExternalInput").ap()
    out_ext = nc.dram_tensor("out", [S, D], F32, kind="ExternalOutput").ap()

    with tile.TileContext(nc) as tc, ExitStack() as ctx:
        # persistent pools
        cst = ctx.enter_context(tc.tile_pool(name="cst", bufs=1))
        wp = ctx.enter_context(tc.tile_pool(name="wp", bufs=1))
        actp = ctx.enter_context(tc.tile_pool(name="actp", bufs=1))
        xs = ctx.enter_context(tc.tile_pool(name="xs", bufs=1))
        pp = ctx.enter_context(tc.tile_pool(name="pp", bufs=3))
        sm = ctx.enter_context(tc.tile_pool(name="sm", bufs=2))
        ob = ctx.enter_context(tc.tile_pool(name="ob", bufs=2))
        ps = ctx.enter_context(tc.tile_pool(name="ps", bufs=2, space="PSUM"))

        def body():
            # ---- constants / weights ----
            ones_f = cst.tile([128, 128], F32, tag="ones_f")
            nc.vector.memset(ones_f[:], 1.0)
            ones_r = cst.tile([128, 128], F32R, tag="ones_r")
            nc.vector.tensor_copy(ones_r[:], ones_f[:])

            bq_sb = cst.tile([128, 2], F32, tag="bq_sb")
            bk_sb = cst.tile([128, 2], F32, tag="bk_sb")
            for i in range(2):
                nc.sync.dma_start(bq_sb[:, i:i + 1], bq_ext[i * 128:(i + 1) * 128, :])
                nc.sync.dma_start(bk_sb[:, i:i + 1], bk_ext[i * 128:(i + 1) * 128, :])
            bv_sb = cst.tile([1, GH], F32R, tag="bv_sb")
            nc.sync.dma_start(bv_sb[:], bv_ext[:])

            wq_sb = wp.tile([128, ND * GH], F32R, tag="wq_sb")
            wk_sb = wp.tile([128, ND * GH], F32R, tag="wk_sb")
            wv_sb = wp.tile([128, ND * GH], F32R, tag="wv_sb")
            for dt_ in range(ND):
                sl = slice(dt_ * GH, (dt_ + 1) * GH)
                nc.sync.dma_start(wq_sb[:, sl], wqT_ext[dt_ * 128:(dt_ + 1) * 128, :])
                nc.sync.dma_start(wk_sb[:, sl], wkT_ext[dt_ * 128:(dt_ + 1) * 128, :])
                nc.sync.dma_start(wv_sb[:, sl], wvT_ext[dt_ * 128:(dt_ + 1) * 128, :])
            wo_sb = wp.tile([128, 2 * D], F32R, tag="wo_sb")
            nc.sync.dma_start(wo_sb[:, 0:D], woT_ext[0:128, :])
            nc.sync.dma_start(wo_sb[:, D:2 * D], woT_ext[128:256, :])

            # ---- projections: q, k -> qhT/khT [2 x [128, S]] ----
            qhT = [actp.tile([128, S], F32R, tag=f"qhT{i}", name=f"qhT{i}") for i in range(2)]
            khT = [actp.tile([128, S], F32R, tag=f"khT{i}", name=f"khT{i}") for i in range(2)]

            for x_ext, w_sb, b_sb, dst in ((qT_ext, wq_sb, bq_sb, qhT),
                                           (kT_ext, wk_sb, bk_sb, khT)):
                for half in range(2):
                    hs = slice(half * 1024, (half + 1) * 1024)
                    xh = xs.tile([128, ND * 1024], F32R, tag="xh")
                    for dt_ in range(ND):
                        nc.sync.dma_start(
                            xh[:, dt_ * 1024:(dt_ + 1) * 1024],
                            x_ext[dt_ * 128:(dt_ + 1) * 128, hs])
                    for sc in range(4):
                        for i in range(2):
                            acc = ps.tile([128, QC], F32, tag="big")
                            for dt_ in range(ND):
                                nc.tensor.matmul(
                                    acc[:],
                                    w_sb[:, dt_ * GH + i * 128:
                                         dt_ * GH + (i + 1) * 128],
                                    xh[:, dt_ * 1024 + sc * QC:
                                       dt_ * 1024 + (sc + 1) * QC],
                                    start=(dt_ == 0), stop=(dt_ == ND - 1))
                            nc.vector.tensor_scalar_add(
                                dst[i][:, half * 1024 + sc * QC:
                                       half * 1024 + (sc + 1) * QC],
                                acc[:], b_sb[:, i:i + 1])

            # ---- projection: v -> vh_aug tiles [128, 512] per k-tile ----
            # head h at cols h*128 : [vh 64 | ones 64]
            vh = [actp.tile([128, 4 * 128], F32R, tag=f"vh{t}", name=f"vh{t}")
                  for t in range(NT)]
            for t in range(NT):
                dst4 = vh[t][:].rearrange("p (h c) -> p h c", h=4)
                nc.vector.tensor_copy(
                    dst4[:, :, 64:128],
                    ones_r[:, 0:64].unsqueeze(1).broadcast_to((128, 4, 64)))
            for half in range(2):
                hs = slice(half * 1024, (half + 1) * 1024)
                vht = xs.tile([128, ND * 1024], F32R, tag="xh")
                for dt_ in range(ND):
                    nc.sync.dma_start(
                        vht[:, dt_ * 1024:(dt_ + 1) * 1024],
                        vT_ext[dt_ * 128:(dt_ + 1) * 128, hs])
                for st8 in range(8):
                    t = half * 8 + st8
                    acc = ps.tile([128, GH], F32, tag="big")
                    for dt_ in range(ND):
                        nc.tensor.matmul(
                            acc[:],
                            vht[:, dt_ * 1024 + st8 * 128:
                                dt_ * 1024 + (st8 + 1) * 128],
                            wv_sb[:, dt_ * GH:(dt_ + 1) * GH],
                            start=(dt_ == 0), stop=False)
                    nc.tensor.matmul(acc[:], ones_r[0:1, 0:128], bv_sb[:],
                                     start=False, stop=True)
                    nc.vector.tensor_copy(
                        vh[t][:].rearrange("p (h c) -> p h c", h=4)[:, :, 0:64],
                        acc[:].rearrange("p (h c) -> p h c", h=4))

            # ---- attention ----
            # stacked normalized ctx^T per pair: [128, S] (A rows 0-63 etc.)
            ctxT = [actp.tile([128, S], F32R, tag=f"ctxT{pr}", name=f"ctxT{pr}")
                    for pr in range(2)]

            st = ps.tile([128, 2048], F32, tag="big", name="st")  # 4 banks, h*512
            for qc in range(NQC):
                qsl = slice(qc * QC, (qc + 1) * QC)
                ctx_ps = ps.tile([128, 2048], F32, tag="big", name="ctx_ps")
                for t in range(NT):
                    for h in range(4):
                        pr, hh = divmod(h, 2)
                        nc.tensor.matmul(
                            st[:, h * 512:h * 512 + QC],
                            khT[pr][hh * 64:(hh + 1) * 64,
                                    t * 128:(t + 1) * 128],
                            qhT[pr][hh * 64:(hh + 1) * 64, qsl],
                            start=True, stop=True)
                    p_sb = pp.tile([128, 1024], F32R, tag="p_sb")
                    nc.scalar.activation(
                        p_sb[:].rearrange("p (h c) -> p h c", h=4),
                        st[:].rearrange("p (h c) -> p h c", h=4)[:, :, 0:QC],
                        EXP)
                    for h in range(4):
                        nc.tensor.matmul(
                            ctx_ps[:, h * 512:h * 512 + QC],
                            vh[t][:, h * 128:(h + 1) * 128],
                            p_sb[:, h * QC:(h + 1) * QC],
                            start=(t == 0), stop=(t == NT - 1))

                # normalize: den rows 64-127 -> shift -> recip -> TT
                den_sb = sm.tile([128, 1024], F32, tag="den_sb")
                nc.vector.tensor_copy(
                    den_sb[64:128, :].rearrange("p (h c) -> p h c", h=4),
                    ctx_ps[:].rearrange("p (h c) -> p h c", h=4)[64:128, :, 0:QC])
                den_lo = sm.tile([128, 1024], F32, tag="den_lo")
                nc.sync.dma_start(den_lo[0:64, :], den_sb[64:128, :])
                recip = sm.tile([128, 1024], F32, tag="recip")
                nc.vector.reciprocal_approx_fast(recip[0:64, :], den_lo[0:64, :])
                bd = sm.tile([128, 512], F32R, tag="bd")
                for h in range(4):
                    pr, hh = divmod(h, 2)
                    if hh == 0:
                        out_ap = ctxT[pr][0:64, qsl]
                    else:
                        out_ap = bd[0:64, pr * QC:(pr + 1) * QC]
                    nc.vector.tensor_mul(
                        out_ap, ctx_ps[0:64, h * 512:h * 512 + QC],
                        recip[0:64, h * QC:(h + 1) * QC])
                for pr in range(2):
                    nc.sync.dma_start(ctxT[pr][64:128, qsl],
                                      bd[0:64, pr * QC:(pr + 1) * QC])

            # ---- output projection ----
            for s_t in range(NT):
                op = ps.tile([128, 2048], F32, tag="big", name="op")
                for n in range(4):
                    for pr in range(2):
                        nc.tensor.matmul(
                            op[:, n * 512:n * 512 + QC],
                            ctxT[pr][:, s_t * 128:(s_t + 1) * 128],
                            wo_sb[:, pr * D + n * QC:pr * D + (n + 1) * QC],
                            start=(pr == 0), stop=(pr == 1))
                o_sb = ob.tile([128, D], F32, tag="o_sb")
                nc.vector.tensor_copy(
                    o_sb[:].rearrange("p (n c) -> p n c", n=4),
                    op[:].rearrange("p (n c) -> p n c", n=4)[:, :, 0:QC])
                nc.sync.dma_start(out_ext[s_t * 128:(s_t + 1) * 128, :],
                                  o_sb[:])

        if loop_r > 1:
            with tc.For_i(0, loop_r, 1):
                body()
        else:
            body()

    nc.compile()
    return nc


class _Runner:
    """SPMD runner on 8 cores via the axon PJRT path (no re-trace)."""

    def __init__(self, nc, n_cores):
        import jax
        from jax.sharding import Mesh, PartitionSpec
        from jax.experimental.shard_map import shard_map
        import concourse.mybir as mybir
        from concourse import bass2jax

        bass2jax.install_neuronx_cc_hook()
        self._jax = jax
        pname = nc.partition_id_tensor.name if nc.partition_id_tensor else None
        in_names, out_names, out_avals, zero_outs = [], [], [], []
        for alloc in nc.m.functions[0].allocations:
            if not isinstance(alloc, mybir.MemoryLocationSet):
                continue
            name = alloc.memorylocations[0].name
            if alloc.kind == "ExternalInput":
                if name != pname:
                    in_names.append(name)
            elif alloc.kind == "ExternalOutput":
                shape = tuple(alloc.tensor_shape)
                dtype = mybir.dt.np(alloc.dtype)
                out_names.append(name)
                out_avals.append(jax.core.ShapedArray(shape, dtype))
                zero_outs.append(np.zeros(shape, dtype))
        self.in_names, self.out_names = in_names, out_names
        self.out_avals, self.zero_outs = out_avals, zero_outs
        self.n_cores = n_cores
        all_in = list(in_names) + list(out_names) + ([pname] if pname else [])

        def _body(*args):
            operands = list(args)
            if pname is not None:
                operands.append(bass2jax.partition_id_tensor())
            return tuple(bass2jax._bass_exec_p.bind(
                *operands, out_avals=tuple(out_avals), in_names=tuple(all_in),
                out_names=tuple(out_names), lowering_input_output_aliases=(),
                sim_require_finite=True, sim_require_nnan=True, nc=nc))

        devices = jax.devices()[:n_cores]
        assert len(devices) >= 1
        self.mesh = Mesh(np.asarray(devices), ("core",))
        spec = PartitionSpec("core")
        n_args = len(in_names) + len(out_names)
        self.fn = jax.jit(
            shard_map(_body, mesh=self.mesh, in_specs=(spec,) * n_args,
                      out_specs=(spec,) * len(out_names), check_rep=False),
            keep_unused=True)
        self.sharding = jax.sharding.NamedSharding(self.mesh, spec)

    def put_inputs(self, in_maps):
        jax = self._jax
        args = []
        for name in self.in_names:
            cat = np.concatenate([np.ascontiguousarray(m[name])
                                  for m in in_maps], axis=0)
            args.append(jax.device_put(cat, self.sharding))
        for z in self.zero_outs:
            cat = np.zeros((self.n_cores * z.shape[0], *z.shape[1:]), z.dtype)
            args.append(jax.device_put(cat, self.sharding))
        return args

    def run(self, args):
        outs = self.fn(*args)
        self._jax.block_until_ready(outs)
        return outs

    def results(self, outs):
        res = []
        for c in range(self.n_cores):
            d = {}
            for i, name in enumerate(self.out_names):
                d[name] = np.asarray(outs[i]).reshape(
                    self.n_cores, *self.out_avals[i].shape)[c]
            res.append(d)
        return res


def _make_in_maps(q, k, v, wq, bq, wk, bk, wv, bv, wo):
    """Host-side sharding/layout prep. Core c = b*4 + g."""
    scale = 1.0 / math.sqrt(DK)
    wq_s = (wq * scale).astype(np.float32)
    bq_s = (bq * scale).astype(np.float32)
    xT = {}
    for b in range(B):
        xT["q", b] = np.ascontiguousarray(q[b].T)
        xT["k", b] = np.ascontiguousarray(k[b].T)
        xT["v", b] = np.ascontiguousarray(v[b].T)
    in_maps = []
    for c in range(NCORES):
        b, g = divmod(c, HPC)
        hd = slice(g * GH, (g + 1) * GH)
        in_maps.append({
            "qT": xT["q", b],
            "kT": xT["k", b],
            "vT": xT["v", b],
            "wqT": np.ascontiguousarray(wq_s[hd, :].T),
            "wkT": np.ascontiguousarray(wk[hd, :].T),
            "wvT": np.ascontiguousarray(wv[hd, :].T),
            "woT": np.ascontiguousarray(wo[:, hd].T),
            "bq": np.ascontiguousarray(bq_s[hd].reshape(GH, 1)),
            "bk": np.ascontiguousarray(bk[hd].reshape(GH, 1)),
            "bv": np.ascontiguousarray(bv[hd].reshape(1, GH)),
        })
    return in_maps


def _numpy_reference(q, k, v, mask, wq, bq, wk, bk, wv, bv, wo, bo):
    """Exact fp32 fallback (only used if mask has zeros)."""
    qh = (q @ wq.T + bq).reshape(B, S, H, DK).transpose(0, 2, 1, 3)
    kh = (k @ wk.T + bk).reshape(B, S, H, DK).transpose(0, 2, 1, 3)
    vh = (v @ wv.T + bv).reshape(B, S, H, DK).transpose(0, 2, 1, 3)
    out = np.zeros((B, S, D), np.float32)
    for b in range(B):
        for h in range(H):
            sc = (qh[b, h] @ kh[b, h].T) / math.sqrt(DK)
            sc = np.where(mask[0, 0] == 0, np.float32(-1e9), sc)
            sc = sc - sc.max(axis=-1, keepdims=True)
            e = np.exp(sc)
            p = e / e.sum(axis=-1, keepdims=True)
            out[b, :, h * DK:(h + 1) * DK] = p @ vh[b, h]
    return out.reshape(B * S, D) @ wo.T + bo


def get_runner(loop_r=1):
    key = ("runner", loop_r)
    if key not in _STATE:
        nc = _build(loop_r=loop_r)
        _STATE[key] = _Runner(nc, NCORES)
    return _STATE[key]


def kernel(q, k, v, mask, wq, bq, wk, bk, wv, bv, wo, bo):
    q = np.asarray(q, np.float32)
    k = np.asarray(k, np.float32)
    v = np.asarray(v, np.float32)
    mask = np.asarray(mask)
    wq = np.asarray(wq, np.float32); bq = np.asarray(bq, np.float32)
    wk = np.asarray(wk, np.float32); bk = np.asarray(bk, np.float32)
    wv = np.asarray(wv, np.float32); bv = np.asarray(bv, np.float32)
    wo = np.asarray(wo, np.float32); bo = np.asarray(bo, np.float32)

    if np.any(mask == 0):
        out = _numpy_reference(q, k, v, mask, wq, bq, wk, bk, wv, bv, wo, bo)
        return out.reshape(B, S, D).astype(np.float32)

    r = get_runner()
    in_maps = _make_in_maps(q, k, v, wq, bq, wk, bk, wv, bv, wo)
    outs = r.run(r.put_inputs(in_maps))
    res = r.results(outs)
    full = np.zeros((B, S, D), np.float32)
    for c in range(NCORES):
        b = c // HPC
        full[b] += res[c]["out"]
    full += bo[None, None, :]
    return full


# revision 10
# speedup vs baseline: 1.3596x; 1.3596x over previous
"""MultiHeadAttention forward on 8 Trainium2 NeuronCores (Bass/Tile).

Problem: B=2, S=2048, D=1024, H=16 heads (dk=64), fp32, mask all-ones.

Sharding: core c = b*4 + g handles batch b and head group g (4 heads).
Data parallel over B, tensor parallel over heads; w_o row-wise with the
partial-output reduction done host-side (summing 4 fp32 partials).

Device kernel per core (all matmuls in float32r = full-rate fp32):
  1. projections: qhT/khT = (w q)^T layouts [256, 2048] (head dim on
     partitions), vh natural [s, dv] per k-tile, biases fused.
  2. attention per q-chunk of 256: scores k-major [k, q] via row-packed
     K=64 head pairs; exp on ScalarE (PSUM -> SBUF, strided over 4
     half-used banks); PV with stationary [vh | ones] so the softmax
     denominator lands replicated on partitions 64-127 of the ctx bank.
  3. normalize: den -> DMA partition shift -> reciprocal_approx -> TT mul,
     writing the stacked ctx^T tiles used as the output-proj stationary.
  4. output projection -> partial out [2048, 1024] per core.

Host: shards/transposes inputs, runs SPMD over 8 cores, sums group
partials per batch, adds bo.
"""
import math

import numpy as np

B, S, D, H = 2, 2048, 1024, 16
DK = D // H          # 64
HPC = H // 4         # 4 heads per core
NCORES = 8
NT = S // 128        # 16 k-tiles / s-tiles
ND = D // 128        # 8 d-tiles
QC = 256             # q-chunk (f32r moving-operand limit)
NQC = S // QC        # 8
GH = HPC * DK        # 256 output dims per group

_STATE = {}


def _build(loop_r=1, parts=('proj', 'attn', 'out')):
    """Build the Bass program (shared by all 8 cores; inputs differ)."""
    from contextlib import ExitStack

    import concourse.tile as tile
    from concourse import bacc, mybir

    F32 = mybir.dt.float32
    F32R = mybir.dt.float32r
    EXP = mybir.ActivationFunctionType.Exp

    nc = bacc.Bacc("TRN2", target_bir_lowering=False, debug=False,
                   num_devices=NCORES)

    qT_ext = nc.dram_tensor("qT", [D, S], F32R, kind="ExternalInput").ap()
    kT_ext = nc.dram_tensor("kT", [D, S], F32R, kind="ExternalInput").ap()
    vT_ext = nc.dram_tensor("vT", [D, S], F32R, kind="ExternalInput").ap()
    wqT_ext = nc.dram_tensor("wqT", [D, GH], F32R, kind="ExternalInput").ap()
    wkT_ext = nc.dram_tensor("wkT", [D, GH], F32R, kind="ExternalInput").ap()
    wvT_ext = nc.dram_tensor("wvT", [D, GH], F32R, kind="ExternalInput").ap()
    woT_ext = nc.dram_tensor("woT", [GH, D], F32R, kind="ExternalInput").ap()
    bq_ext = nc.dram_tensor("bq", [GH, 1], F32, kind="ExternalInput").ap()
    bk_ext = nc.dram_tensor("bk", [GH, 1], F32, kind="ExternalInput").ap()
    bv_ext = nc.dram_tensor("bv", [1, GH], F32R, kind="ExternalInput").ap()
    out_ext = nc.dram_tensor("out", [S, D], F32, kind="ExternalOutput").ap()

    with tile.TileContext(nc) as tc, ExitStack() as ctx:
        # persistent pools
        cst = ctx.enter_context(tc.tile_pool(name="cst", bufs=1))
        wp = ctx.enter_context(tc.tile_pool(name="wp", bufs=1))
        actp = ctx.enter_context(tc.tile_pool(name="actp", bufs=1))
        xs = ctx.enter_context(tc.tile_pool(name="xs", bufs=2))
        pp = ctx.enter_context(tc.tile_pool(name="pp", bufs=3))
        sm = ctx.enter_context(tc.tile_pool(name="sm", bufs=1))
        ob = ctx.enter_context(tc.tile_pool(name="ob", bufs=2))
        ps = ctx.enter_context(tc.tile_pool(name="ps", bufs=1, space="PSUM"))

        def body():
            # ---- constants / weights ----
            ones_f = cst.tile([128, 128], F32, tag="ones_f")
            nc.vector.memset(ones_f[:], 1.0)
            ones_r = cst.tile([128, 128], F32R, tag="ones_r")
            nc.vector.tensor_copy(ones_r[:], ones_f[:])

            bq_sb = cst.tile([128, 2], F32, tag="bq_sb")
            bk_sb = cst.tile([128, 2], F32, tag="bk_sb")
            for i in range(2):
                nc.sync.dma_start(bq_sb[:, i:i + 1], bq_ext[i * 128:(i + 1) * 128, :])
                nc.sync.dma_start(bk_sb[:, i:i + 1], bk_ext[i * 128:(i + 1) * 128, :])
            bv_sb = cst.tile([1, GH], F32R, tag="bv_sb")
            nc.sync.dma_start(bv_sb[:], bv_ext[:])

            wq_sb = wp.tile([128, ND * GH], F32R, tag="wq_sb")
            wk_sb = wp.tile([128, ND * GH], F32R, tag="wk_sb")
            wv_sb = wp.tile([128, ND * GH], F32R, tag="wv_sb")
            for dt_ in range(ND):
                sl = slice(dt_ * GH, (dt_ + 1) * GH)
                nc.sync.dma_start(wq_sb[:, sl], wqT_ext[dt_ * 128:(dt_ + 1) * 128, :])
                nc.sync.dma_start(wk_sb[:, sl], wkT_ext[dt_ * 128:(dt_ + 1) * 128, :])
                nc.sync.dma_start(wv_sb[:, sl], wvT_ext[dt_ * 128:(dt_ + 1) * 128, :])
            wo_sb = wp.tile([128, 2 * D], F32R, tag="wo_sb")
            nc.sync.dma_start(wo_sb[:, 0:D], woT_ext[0:128, :])
            nc.sync.dma_start(wo_sb[:, D:2 * D], woT_ext[128:256, :])

            # ---- projections: q, k -> qhT/khT [2 x [128, S]] ----
            qhT = [actp.tile([128, S], F32R, tag=f"qhT{i}", name=f"qhT{i}") for i in range(2)]
            khT = [actp.tile([128, S], F32R, tag=f"khT{i}", name=f"khT{i}") for i in range(2)]

            for x_ext, w_sb, b_sb, dst in ((qT_ext, wq_sb, bq_sb, qhT),
                                           (kT_ext, wk_sb, bk_sb, khT)):
                for qtr in range(4):
                    hs = slice(qtr * 512, (qtr + 1) * 512)
                    xh = xs.tile([128, ND * 512], F32R, tag="xh")
                    for dt_ in range(ND):
                        nc.sync.dma_start(
                            xh[:, dt_ * 512:(dt_ + 1) * 512],
                            x_ext[dt_ * 128:(dt_ + 1) * 128, hs])
                    for sc in range(2):
                        for i in range(2):
                            acc = ps.tile([128, QC], F32,
                                          tag=("stA" if (sc * 2 + i) % 2 == 0
                                               else "stB"))
                            for dt_ in range(ND):
                                nc.tensor.matmul(
                                    acc[:],
                                    w_sb[:, dt_ * GH + i * 128:
                                         dt_ * GH + (i + 1) * 128],
                                    xh[:, dt_ * 512 + sc * QC:
                                       dt_ * 512 + (sc + 1) * QC],
                                    start=(dt_ == 0), stop=(dt_ == ND - 1))
                            nc.vector.tensor_scalar_add(
                                dst[i][:, qtr * 512 + sc * QC:
                                       qtr * 512 + (sc + 1) * QC],
                                acc[:], b_sb[:, i:i + 1])

            # ---- projection: v -> vh_aug tiles [128, 512] per k-tile ----
            # head h at cols h*128 : [vh 64 | ones 64]
            vh = [actp.tile([128, 4 * 128], F32R, tag=f"vh{t}", name=f"vh{t}")
                  for t in range(NT)]
            for t in range(NT):
                dst4 = vh[t][:].rearrange("p (h c) -> p h c", h=4)
                nc.vector.tensor_copy(
                    dst4[:, :, 64:128],
                    ones_r[:, 0:64].unsqueeze(1).broadcast_to((128, 4, 64)))
            for qtr in range(4):
                hs = slice(qtr * 512, (qtr + 1) * 512)
                vht = xs.tile([128, ND * 512], F32R, tag="xh")
                for dt_ in range(ND):
                    nc.sync.dma_start(
                        vht[:, dt_ * 512:(dt_ + 1) * 512],
                        vT_ext[dt_ * 128:(dt_ + 1) * 128, hs])
                for st8 in range(4):
                    t = qtr * 4 + st8
                    acc = ps.tile([128, GH], F32,
                                  tag=("stA" if st8 % 2 == 0 else "stB"))
                    for dt_ in range(ND):
                        nc.tensor.matmul(
                            acc[:],
                            vht[:, dt_ * 512 + st8 * 128:
                                dt_ * 512 + (st8 + 1) * 128],
                            wv_sb[:, dt_ * GH:(dt_ + 1) * GH],
                            start=(dt_ == 0), stop=False)
                    nc.tensor.matmul(acc[:], ones_r[0:1, 0:128], bv_sb[:],
                                     start=False, stop=True)
                    nc.vector.tensor_copy(
                        vh[t][:].rearrange("p (h c) -> p h c", h=4)[:, :, 0:64],
                        acc[:].rearrange("p (h c) -> p h c", h=4))

            if 'attn' not in parts:
                # drain: touch outputs so they're written
                o_sb0 = ob.tile([128, D], F32, tag="o_sb")
                nc.vector.tensor_copy(o_sb0[:, 0:S // NT], qhT[0][:, 0:S // NT])
                nc.sync.dma_start(out_ext[0:128, :], o_sb0[:])
                return
            # ---- attention ----
            # stacked normalized ctx^T per pair: [128, S] (A rows 0-63 etc.)
            ctxT = [actp.tile([128, S], F32R, tag=f"ctxT{pr}", name=f"ctxT{pr}")
                    for pr in range(2)]

            # stA holds heads 0,1 (pair 0); stB heads 2,3 — ping-pong with ACT
            stA = ps.tile([128, 1024], F32, tag="stA", name="stA")
            stB = ps.tile([128, 1024], F32, tag="stB", name="stB")
            sts = (stA, stB)

            def scores(pr, t, qsl):
                for hh in range(2):
                    nc.tensor.matmul(
                        sts[pr][:, hh * 512:hh * 512 + QC],
                        khT[pr][hh * 64:(hh + 1) * 64, t * 128:(t + 1) * 128],
                        qhT[pr][hh * 64:(hh + 1) * 64, qsl],
                        start=True, stop=True)

            for qc in range(NQC):
                qsl = slice(qc * QC, (qc + 1) * QC)
                ctx_ps = ps.tile([128, 2048], F32, tag="ctx", name="ctx_ps")
                scores(0, 0, qsl)
                scores(1, 0, qsl)
                for t in range(NT):
                    ps_t = [None, None]
                    for pr in range(2):
                        p_sb = pp.tile([128, 512], F32R, tag=f"p{pr}",
                                       name=f"p{pr}")
                        nc.scalar.activation(
                            p_sb[:].rearrange("p (h c) -> p h c", h=2),
                            sts[pr][:].rearrange("p (h c) -> p h c",
                                                 h=2)[:, :, 0:QC],
                            EXP)
                        ps_t[pr] = p_sb
                    if t + 1 < NT:
                        scores(0, t + 1, qsl)
                        scores(1, t + 1, qsl)
                    for h in range(4):
                        pr, hh = divmod(h, 2)
                        nc.tensor.matmul(
                            ctx_ps[:, h * 512:h * 512 + QC],
                            vh[t][:, h * 128:(h + 1) * 128],
                            ps_t[pr][:, hh * QC:(hh + 1) * QC],
                            start=(t == 0), stop=(t == NT - 1))

                # normalize: den rows 64-127 -> shift -> recip -> TT
                den_sb = sm.tile([128, 1024], F32, tag="den_sb")
                nc.vector.tensor_copy(
                    den_sb[64:128, :].rearrange("p (h c) -> p h c", h=4),
                    ctx_ps[:].rearrange("p (h c) -> p h c", h=4)[64:128, :, 0:QC])
                den_lo = sm.tile([128, 1024], F32, tag="den_lo")
                nc.sync.dma_start(den_lo[0:64, :], den_sb[64:128, :])
                recip = sm.tile([128, 1024], F32, tag="recip")
                nc.vector.reciprocal_approx_fast(recip[0:64, :], den_lo[0:64, :])
                bd = sm.tile([128, 512], F32R, tag="bd")
                for h in range(4):
                    pr, hh = divmod(h, 2)
                    if hh == 0:
                        out_ap = ctxT[pr][0:64, qsl]
                    else:
                        out_ap = bd[0:64, pr * QC:(pr + 1) * QC]
                    nc.vector.tensor_mul(
                        out_ap, ctx_ps[0:64, h * 512:h * 512 + QC],
                        recip[0:64, h * QC:(h + 1) * QC])
                for pr in range(2):
                    nc.sync.dma_start(ctxT[pr][64:128, qsl],
                                      bd[0:64, pr * QC:(pr + 1) * QC])

            if 'out' not in parts:
                o_sb0 = ob.tile([128, D], F32, tag="o_sb")
                nc.vector.tensor_copy(o_sb0[:], ctxT[0][:, 0:D])
                nc.sync.dma_start(out_ext[0:128, :], o_sb0[:])
                return
            # ---- output projection ----
            for s_t in range(NT):
                op = ps.tile([128, 2048], F32, tag="ctx", name="op")
                for n in range(4):
                    for pr in range(2):
                        nc.tensor.matmul(
                            op[:, n * 512:n * 512 + QC],
                            ctxT[pr][:, s_t * 128:(s_t + 1) * 128],
                            wo_sb[:, pr * D + n * QC:pr * D + (n + 1) * QC],
                            start=(pr == 0), stop=(pr == 1))
                o_sb = ob.tile([128, D], F32, tag="o_sb")
                nc.vector.tensor_copy(
                    o_sb[:].rearrange("p (n c) -> p n c", n=4),
                    op[:].rearrange("p (n c) -> p n c", n=4)[:, :, 0:QC])
                nc.sync.dma_start(out_ext[s_t * 128:(s_t + 1) * 128, :],
                                  o_sb[:])

        if loop_r > 1:
            with tc.For_i(0, loop_r, 1):
                body()
        else:
            body()

    nc.compile()
    return nc


class _Runner:
    """SPMD runner on 8 cores via the axon PJRT path (no re-trace)."""

    def __init__(self, nc, n_cores):
        import jax
        from jax.sharding import Mesh, PartitionSpec
        from jax.experimental.shard_map import shard_map
        import concourse.mybir as mybir
        from concourse import bass2jax

        bass2jax.install_neuronx_cc_hook()
        self._jax = jax
        pname = nc.partition_id_tensor.name if nc.partition_id_tensor else None
        in_names, out_names, out_avals, zero_outs = [], [], [], []
        for alloc in nc.m.functions[0].allocations:
            if not isinstance(alloc, mybir.MemoryLocationSet):
                continue
            name = alloc.memorylocations[0].name
            if alloc.kind == "ExternalInput":
                if name != pname:
                    in_names.append(name)
            elif alloc.kind == "ExternalOutput":
                shape = tuple(alloc.tensor_shape)
                dtype = mybir.dt.np(alloc.dtype)
                out_names.append(name)
                out_avals.append(jax.core.ShapedArray(shape, dtype))
                zero_outs.append(np.zeros(shape, dtype))
        self.in_names, self.out_names = in_names, out_names
        self.out_avals, self.zero_outs = out_avals, zero_outs
        self.n_cores = n_cores
        all_in = list(in_names) + list(out_names) + ([pname] if pname else [])

        def _body(*args):
            operands = list(args)
            if pname is not None:
                operands.append(bass2jax.partition_id_tensor())
            return tuple(bass2jax._bass_exec_p.bind(
                *operands, out_avals=tuple(out_avals), in_names=tuple(all_in),
                out_names=tuple(out_names), lowering_input_output_aliases=(),
                sim_require_finite=True, sim_require_nnan=True, nc=nc))

        devices = jax.devices()[:n_cores]
        assert len(devices) >= 1
        self.mesh = Mesh(np.asarray(devices), ("core",))
        spec = PartitionSpec("core")
        n_args = len(in_names) + len(out_names)
        self.fn = jax.jit(
            shard_map(_body, mesh=self.mesh, in_specs=(spec,) * n_args,
                      out_specs=(spec,) * len(out_names), check_rep=False),
            keep_unused=True)
        self.sharding = jax.sharding.NamedSharding(self.mesh, spec)

    def put_inputs(self, in_maps):
        jax = self._jax
        args = []
        for name in self.in_names:
            cat = np.concatenate([np.ascontiguousarray(m[name])
                                  for m in in_maps], axis=0)
            args.append(jax.device_put(cat, self.sharding))
        for z in self.zero_outs:
            cat = np.zeros((self.n_cores * z.shape[0], *z.shape[1:]), z.dtype)
            args.append(jax.device_put(cat, self.sharding))
        return args

    def run(self, args):
        outs = self.fn(*args)
        self._jax.block_until_ready(outs)
        return outs

    def results(self, outs):
        res = []
        for c in range(self.n_cores):
            d = {}
            for i, name in enumerate(self.out_names):
                d[name] = np.asarray(outs[i]).reshape(
                    self.n_cores, *self.out_avals[i].shape)[c]
            res.append(d)
        return res


def _make_in_maps(q, k, v, wq, bq, wk, bk, wv, bv, wo):
    """Host-side sharding/layout prep. Core c = b*4 + g."""
    scale = 1.0 / math.sqrt(DK)
    wq_s = (wq * scale).astype(np.float32)
    bq_s = (bq * scale).astype(np.float32)
    xT = {}
    for b in range(B):
        xT["q", b] = np.ascontiguousarray(q[b].T)
        xT["k", b] = np.ascontiguousarray(k[b].T)
        xT["v", b] = np.ascontiguousarray(v[b].T)
    in_maps = []
    for c in range(NCORES):
        b, g = divmod(c, HPC)
        hd = slice(g * GH, (g + 1) * GH)
        in_maps.append({
            "qT": xT["q", b],
            "kT": xT["k", b],
            "vT": xT["v", b],
            "wqT": np.ascontiguousarray(wq_s[hd, :].T),
            "wkT": np.ascontiguousarray(wk[hd, :].T),
            "wvT": np.ascontiguousarray(wv[hd, :].T),
            "woT": np.ascontiguousarray(wo[:, hd].T),
            "bq": np.ascontiguousarray(bq_s[hd].reshape(GH, 1)),
            "bk": np.ascontiguousarray(bk[hd].reshape(GH, 1)),
            "bv": np.ascontiguousarray(bv[hd].reshape(1, GH)),
        })
    return in_maps


def _numpy_reference(q, k, v, mask, wq, bq, wk, bk, wv, bv, wo, bo):
    """Exact fp32 fallback (only used if mask has zeros)."""
    qh = (q @ wq.T + bq).reshape(B, S, H, DK).transpose(0, 2, 1, 3)
    kh = (k @ wk.T + bk).reshape(B, S, H, DK).transpose(0, 2, 1, 3)
    vh = (v @ wv.T + bv).reshape(B, S, H, DK).transpose(0, 2, 1, 3)
    out = np.zeros((B, S, D), np.float32)
    for b in range(B):
        for h in range(H):
            sc = (qh[b, h] @ kh[b, h].T) / math.sqrt(DK)
            sc = np.where(mask[0, 0] == 0, np.float32(-1e9), sc)
            sc = sc - sc.max(axis=-1, keepdims=True)
            e = np.exp(sc)
            p = e / e.sum(axis=-1, keepdims=True)
            out[b, :, h * DK:(h + 1) * DK] = p @ vh[b, h]
    return out.reshape(B * S, D) @ wo.T + bo


def get_runner(loop_r=1, parts=('proj', 'attn', 'out')):
    key = ("runner", loop_r, tuple(parts))
    if key not in _STATE:
        nc = _build(loop_r=loop_r, parts=parts)
        _STATE[key] = _Runner(nc, NCORES)
    return _STATE[key]


def kernel(q, k, v, mask, wq, bq, wk, bk, wv, bv, wo, bo):
    q = np.asarray(q, np.float32)
    k = np.asarray(k, np.float32)
    v = np.asarray(v, np.float32)
    mask = np.asarray(mask)
    wq = np.asarray(wq, np.float32); bq = np.asarray(bq, np.float32)
    wk = np.asarray(wk, np.float32); bk = np.asarray(bk, np.float32)
    wv = np.asarray(wv, np.float32); bv = np.asarray(bv, np.float32)
    wo = np.asarray(wo, np.float32); bo = np.asarray(bo, np.float32)

    if np.any(mask == 0):
        out = _numpy_reference(q, k, v, mask, wq, bq, wk, bk, wv, bv, wo, bo)
        return out.reshape(B, S, D).astype(np.float32)

    r = get_runner()
    in_maps = _make_in_maps(q, k, v, wq, bq, wk, bk, wv, bv, wo)
    outs = r.run(r.put_inputs(in_maps))
    res = r.results(outs)
    full = np.zeros((B, S, D), np.float32)
    for c in range(NCORES):
        b = c // HPC
        full[b] += res[c]["out"]
    full += bo[None, None, :]
    return full


# revision 11
# speedup vs baseline: 1.3855x; 1.0191x over previous
"""MultiHeadAttention forward on 8 Trainium2 NeuronCores (Bass/Tile).

Problem: B=2, S=2048, D=1024, H=16 heads (dk=64), fp32, mask all-ones.

Sharding: core c = b*4 + g handles batch b and head group g (4 heads).
Data parallel over B, tensor parallel over heads; w_o row-wise with the
partial-output reduction done host-side (summing 4 fp32 partials).

Device kernel per core (all matmuls in float32r = full-rate fp32):
  1. projections: qhT/khT = (w q)^T layouts [256, 2048] (head dim on
     partitions), vh natural [s, dv] per k-tile, biases fused.
  2. attention per q-chunk of 256: scores k-major [k, q] via row-packed
     K=64 head pairs; exp on ScalarE (PSUM -> SBUF, strided over 4
     half-used banks); PV with stationary [vh | ones] so the softmax
     denominator lands replicated on partitions 64-127 of the ctx bank.
  3. normalize: den -> DMA partition shift -> reciprocal_approx -> TT mul,
     writing the stacked ctx^T tiles used as the output-proj stationary.
  4. output projection -> partial out [2048, 1024] per core.

Host: shards/transposes inputs, runs SPMD over 8 cores, sums group
partials per batch, adds bo.
"""
import math

import numpy as np

B, S, D, H = 2, 2048, 1024, 16
DK = D // H          # 64
HPC = H // 4         # 4 heads per core
NCORES = 8
NT = S // 128        # 16 k-tiles / s-tiles
ND = D // 128        # 8 d-tiles
QC = 256             # q-chunk (f32r moving-operand limit)
NQC = S // QC        # 8
GH = HPC * DK        # 256 output dims per group

_STATE = {}


def _build(loop_r=1, parts=('proj', 'attn', 'out')):
    """Build the Bass program (shared by all 8 cores; inputs differ)."""
    from contextlib import ExitStack

    import concourse.tile as tile
    from concourse import bacc, mybir

    F32 = mybir.dt.float32
    F32R = mybir.dt.float32r
    EXP = mybir.ActivationFunctionType.Exp

    nc = bacc.Bacc("TRN2", target_bir_lowering=False, debug=False,
                   num_devices=NCORES)

    qT_ext = nc.dram_tensor("qT", [D, S], F32R, kind="ExternalInput").ap()
    kT_ext = nc.dram_tensor("kT", [D, S], F32R, kind="ExternalInput").ap()
    vT_ext = nc.dram_tensor("vT", [D, S], F32R, kind="ExternalInput").ap()
    wqT_ext = nc.dram_tensor("wqT", [D, GH], F32R, kind="ExternalInput").ap()
    wkT_ext = nc.dram_tensor("wkT", [D, GH], F32R, kind="ExternalInput").ap()
    wvT_ext = nc.dram_tensor("wvT", [D, GH], F32R, kind="ExternalInput").ap()
    woT_ext = nc.dram_tensor("woT", [GH, D], F32R, kind="ExternalInput").ap()
    bq_ext = nc.dram_tensor("bq", [GH, 1], F32, kind="ExternalInput").ap()
    bk_ext = nc.dram_tensor("bk", [GH, 1], F32, kind="ExternalInput").ap()
    bv_ext = nc.dram_tensor("bv", [1, GH], F32R, kind="ExternalInput").ap()
    out_ext = nc.dram_tensor("out", [S, D], F32, kind="ExternalOutput").ap()

    with tile.TileContext(nc) as tc, ExitStack() as ctx:
        # persistent pools
        cst = ctx.enter_context(tc.tile_pool(name="cst", bufs=1))
        wp = ctx.enter_context(tc.tile_pool(name="wp", bufs=1))
        actp = ctx.enter_context(tc.tile_pool(name="actp", bufs=1))
        xs = ctx.enter_context(tc.tile_pool(name="xs", bufs=2))
        pp = ctx.enter_context(tc.tile_pool(name="pp", bufs=3))
        sm = ctx.enter_context(tc.tile_pool(name="sm", bufs=1))
        ob = ctx.enter_context(tc.tile_pool(name="ob", bufs=2))
        ps = ctx.enter_context(tc.tile_pool(name="ps", bufs=1, space="PSUM"))

        def body():
            # ---- constants / weights ----
            ones_f = cst.tile([128, 128], F32, tag="ones_f")
            nc.vector.memset(ones_f[:], 1.0)
            ones_r = cst.tile([128, 128], F32R, tag="ones_r")
            nc.vector.tensor_copy(ones_r[:], ones_f[:])

            bq_sb = cst.tile([128, 2], F32, tag="bq_sb")
            bk_sb = cst.tile([128, 2], F32, tag="bk_sb")
            for i in range(2):
                nc.sync.dma_start(bq_sb[:, i:i + 1], bq_ext[i * 128:(i + 1) * 128, :])
                nc.sync.dma_start(bk_sb[:, i:i + 1], bk_ext[i * 128:(i + 1) * 128, :])
            bv_sb = cst.tile([1, GH], F32R, tag="bv_sb")
            nc.sync.dma_start(bv_sb[:], bv_ext[:])

            wq_sb = wp.tile([128, ND * GH], F32R, tag="wq_sb")
            wk_sb = wp.tile([128, ND * GH], F32R, tag="wk_sb")
            wv_sb = wp.tile([128, ND * GH], F32R, tag="wv_sb")
            for dt_ in range(ND):
                sl = slice(dt_ * GH, (dt_ + 1) * GH)
                nc.sync.dma_start(wq_sb[:, sl], wqT_ext[dt_ * 128:(dt_ + 1) * 128, :])
                nc.sync.dma_start(wk_sb[:, sl], wkT_ext[dt_ * 128:(dt_ + 1) * 128, :])
                nc.sync.dma_start(wv_sb[:, sl], wvT_ext[dt_ * 128:(dt_ + 1) * 128, :])
            wo_sb = wp.tile([128, 2 * D], F32R, tag="wo_sb")
            nc.sync.dma_start(wo_sb[:, 0:D], woT_ext[0:128, :])
            nc.sync.dma_start(wo_sb[:, D:2 * D], woT_ext[128:256, :])

            # ---- projections: q, k -> qhT/khT [2 x [128, S]] ----
            qhT = [actp.tile([128, S], F32R, tag=f"qhT{i}", name=f"qhT{i}") for i in range(2)]
            khT = [actp.tile([128, S], F32R, tag=f"khT{i}", name=f"khT{i}") for i in range(2)]

            for x_ext, w_sb, b_sb, dst in ((qT_ext, wq_sb, bq_sb, qhT),
                                           (kT_ext, wk_sb, bk_sb, khT)):
                xv = x_ext.rearrange("(a p) s -> p a s", p=128)
                for qtr in range(4):
                    hs = slice(qtr * 512, (qtr + 1) * 512)
                    xh = xs.tile([128, ND * 512], F32R, tag="xh")
                    nc.sync.dma_start(
                        xh[:].rearrange("p (a s) -> p a s", a=ND),
                        xv[:, :, hs])
                    for sc in range(2):
                        for i in range(2):
                            acc = ps.tile([128, QC], F32,
                                          tag=("stA" if (sc * 2 + i) % 2 == 0
                                               else "stB"))
                            for dt_ in range(ND):
                                nc.tensor.matmul(
                                    acc[:],
                                    w_sb[:, dt_ * GH + i * 128:
                                         dt_ * GH + (i + 1) * 128],
                                    xh[:, dt_ * 512 + sc * QC:
                                       dt_ * 512 + (sc + 1) * QC],
                                    start=(dt_ == 0), stop=(dt_ == ND - 1))
                            nc.vector.tensor_scalar_add(
                                dst[i][:, qtr * 512 + sc * QC:
                                       qtr * 512 + (sc + 1) * QC],
                                acc[:], b_sb[:, i:i + 1])

            # ---- projection: v -> vh_aug tiles [128, 512] per k-tile ----
            # head h at cols h*128 : [vh 64 | ones 64]
            vh = [actp.tile([128, 4 * 128], F32R, tag=f"vh{t}", name=f"vh{t}")
                  for t in range(NT)]
            for t in range(NT):
                dst4 = vh[t][:].rearrange("p (h c) -> p h c", h=4)
                nc.vector.tensor_copy(
                    dst4[:, :, 64:128],
                    ones_r[:, 0:64].unsqueeze(1).broadcast_to((128, 4, 64)))
            vv = vT_ext.rearrange("(a p) s -> p a s", p=128)
            for qtr in range(4):
                hs = slice(qtr * 512, (qtr + 1) * 512)
                vht = xs.tile([128, ND * 512], F32R, tag="xh")
                nc.sync.dma_start(
                    vht[:].rearrange("p (a s) -> p a s", a=ND),
                    vv[:, :, hs])
                for st8 in range(4):
                    t = qtr * 4 + st8
                    acc = ps.tile([128, GH], F32,
                                  tag=("stA" if st8 % 2 == 0 else "stB"))
                    for dt_ in range(ND):
                        nc.tensor.matmul(
                            acc[:],
                            vht[:, dt_ * 512 + st8 * 128:
                                dt_ * 512 + (st8 + 1) * 128],
                            wv_sb[:, dt_ * GH:(dt_ + 1) * GH],
                            start=(dt_ == 0), stop=False)
                    nc.tensor.matmul(acc[:], ones_r[0:1, 0:128], bv_sb[:],
                                     start=False, stop=True)
                    nc.vector.tensor_copy(
                        vh[t][:].rearrange("p (h c) -> p h c", h=4)[:, :, 0:64],
                        acc[:].rearrange("p (h c) -> p h c", h=4))

            if 'attn' not in parts:
                # drain: touch outputs so they're written
                o_sb0 = ob.tile([128, D], F32, tag="o_sb")
                nc.vector.tensor_copy(o_sb0[:, 0:S // NT], qhT[0][:, 0:S // NT])
                nc.sync.dma_start(out_ext[0:128, :], o_sb0[:])
                return
            # ---- attention ----
            # stacked normalized ctx^T per pair: [128, S] (A rows 0-63 etc.)
            ctxT = [actp.tile([128, S], F32R, tag=f"ctxT{pr}", name=f"ctxT{pr}")
                    for pr in range(2)]

            # stA holds heads 0,1 (pair 0); stB heads 2,3 — ping-pong with ACT
            stA = ps.tile([128, 1024], F32, tag="stA", name="stA")
            stB = ps.tile([128, 1024], F32, tag="stB", name="stB")
            sts = (stA, stB)

            def scores(pr, t, qsl):
                for hh in range(2):
                    nc.tensor.matmul(
                        sts[pr][:, hh * 512:hh * 512 + QC],
                        khT[pr][hh * 64:(hh + 1) * 64, t * 128:(t + 1) * 128],
                        qhT[pr][hh * 64:(hh + 1) * 64, qsl],
                        start=True, stop=True)

            for qc in range(NQC):
                qsl = slice(qc * QC, (qc + 1) * QC)
                ctx_ps = ps.tile([128, 2048], F32, tag="ctx", name="ctx_ps")
                scores(0, 0, qsl)
                scores(1, 0, qsl)
                for t in range(NT):
                    ps_t = [None, None]
                    for pr in range(2):
                        p_sb = pp.tile([128, 512], F32R, tag=f"p{pr}",
                                       name=f"p{pr}")
                        nc.scalar.activation(
                            p_sb[:].rearrange("p (h c) -> p h c", h=2),
                            sts[pr][:].rearrange("p (h c) -> p h c",
                                                 h=2)[:, :, 0:QC],
                            EXP)
                        ps_t[pr] = p_sb
                    if t + 1 < NT:
                        scores(0, t + 1, qsl)
                        scores(1, t + 1, qsl)
                    for h in range(4):
                        pr, hh = divmod(h, 2)
                        nc.tensor.matmul(
                            ctx_ps[:, h * 512:h * 512 + QC],
                            vh[t][:, h * 128:(h + 1) * 128],
                            ps_t[pr][:, hh * QC:(hh + 1) * QC],
                            start=(t == 0), stop=(t == NT - 1))

                # normalize: den rows 64-127 -> shift -> recip -> TT
                den_sb = sm.tile([128, 1024], F32, tag="den_sb")
                nc.vector.tensor_copy(
                    den_sb[64:128, :].rearrange("p (h c) -> p h c", h=4),
                    ctx_ps[:].rearrange("p (h c) -> p h c", h=4)[64:128, :, 0:QC])
                den_lo = sm.tile([128, 1024], F32, tag="den_lo")
                nc.sync.dma_start(den_lo[0:64, :], den_sb[64:128, :])
                recip = sm.tile([128, 1024], F32, tag="recip")
                nc.vector.reciprocal_approx_fast(recip[0:64, :], den_lo[0:64, :])
                bd = sm.tile([128, 512], F32R, tag="bd")
                for h in range(4):
                    pr, hh = divmod(h, 2)
                    if hh == 0:
                        out_ap = ctxT[pr][0:64, qsl]
                    else:
                        out_ap = bd[0:64, pr * QC:(pr + 1) * QC]
                    nc.vector.tensor_mul(
                        out_ap, ctx_ps[0:64, h * 512:h * 512 + QC],
                        recip[0:64, h * QC:(h + 1) * QC])
                for pr in range(2):
                    nc.sync.dma_start(ctxT[pr][64:128, qsl],
                                      bd[0:64, pr * QC:(pr + 1) * QC])

            if 'out' not in parts:
                o_sb0 = ob.tile([128, D], F32, tag="o_sb")
                nc.vector.tensor_copy(o_sb0[:], ctxT[0][:, 0:D])
                nc.sync.dma_start(out_ext[0:128, :], o_sb0[:])
                return
            # ---- output projection ----
            for s_t in range(NT):
                o_sb = ob.tile([128, D], F32, tag="o_sb")
                for nh in range(2):
                    op = ps.tile([128, 1024], F32,
                                 tag=("stA" if nh == 0 else "stB"),
                                 name=f"op{nh}")
                    for n2 in range(2):
                        n = nh * 2 + n2
                        for pr in range(2):
                            nc.tensor.matmul(
                                op[:, n2 * 512:n2 * 512 + QC],
                                ctxT[pr][:, s_t * 128:(s_t + 1) * 128],
                                wo_sb[:, pr * D + n * QC:pr * D + (n + 1) * QC],
                                start=(pr == 0), stop=(pr == 1))
                    nc.vector.tensor_copy(
                        o_sb[:, nh * 512:(nh + 1) * 512]
                        .rearrange("p (n c) -> p n c", n=2),
                        op[:].rearrange("p (n c) -> p n c", n=2)[:, :, 0:QC])
                nc.sync.dma_start(out_ext[s_t * 128:(s_t + 1) * 128, :],
                                  o_sb[:])

        if loop_r > 1:
            with tc.For_i(0, loop_r, 1):
                body()
        else:
            body()

    nc.compile()
    return nc


class _Runner:
    """SPMD runner on 8 cores via the axon PJRT path (no re-trace)."""

    def __init__(self, nc, n_cores):
        import jax
        from jax.sharding import Mesh, PartitionSpec
        from jax.experimental.shard_map import shard_map
        import concourse.mybir as mybir
        from concourse import bass2jax

        bass2jax.install_neuronx_cc_hook()
        self._jax = jax
        pname = nc.partition_id_tensor.name if nc.partition_id_tensor else None
        in_names, out_names, out_avals, zero_outs = [], [], [], []
        for alloc in nc.m.functions[0].allocations:
            if not isinstance(alloc, mybir.MemoryLocationSet):
                continue
            name = alloc.memorylocations[0].name
            if alloc.kind == "ExternalInput":
                if name != pname:
                    in_names.append(name)
            elif alloc.kind == "ExternalOutput":
                shape = tuple(alloc.tensor_shape)
                dtype = mybir.dt.np(alloc.dtype)
                out_names.append(name)
                out_avals.append(jax.core.ShapedArray(shape, dtype))
                zero_outs.append(np.zeros(shape, dtype))
        self.in_names, self.out_names = in_names, out_names
        self.out_avals, self.zero_outs = out_avals, zero_outs
        self.n_cores = n_cores
        all_in = list(in_names) + list(out_names) + ([pname] if pname else [])

        def _body(*args):
            operands = list(args)
            if pname is not None:
                operands.append(bass2jax.partition_id_tensor())
            return tuple(bass2jax._bass_exec_p.bind(
                *operands, out_avals=tuple(out_avals), in_names=tuple(all_in),
                out_names=tuple(out_names), lowering_input_output_aliases=(),
                sim_require_finite=True, sim_require_nnan=True, nc=nc))

        devices = jax.devices()[:n_cores]
        assert len(devices) >= 1
        self.mesh = Mesh(np.asarray(devices), ("core",))
        spec = PartitionSpec("core")
        n_args = len(in_names) + len(out_names)
        self.fn = jax.jit(
            shard_map(_body, mesh=self.mesh, in_specs=(spec,) * n_args,
                      out_specs=(spec,) * len(out_names), check_rep=False),
            keep_unused=True)
        self.sharding = jax.sharding.NamedSharding(self.mesh, spec)

    def put_inputs(self, in_maps):
        jax = self._jax
        args = []
        for name in self.in_names:
            cat = np.concatenate([np.ascontiguousarray(m[name])
                                  for m in in_maps], axis=0)
            args.append(jax.device_put(cat, self.sharding))
        for z in self.zero_outs:
            cat = np.zeros((self.n_cores * z.shape[0], *z.shape[1:]), z.dtype)
            args.append(jax.device_put(cat, self.sharding))
        return args

    def run(self, args):
        outs = self.fn(*args)
        self._jax.block_until_ready(outs)
        return outs

    def results(self, outs):
        res = []
        for c in range(self.n_cores):
            d = {}
            for i, name in enumerate(self.out_names):
                d[name] = np.asarray(outs[i]).reshape(
                    self.n_cores, *self.out_avals[i].shape)[c]
            res.append(d)
        return res


def _make_in_maps(q, k, v, wq, bq, wk, bk, wv, bv, wo):
    """Host-side sharding/layout prep. Core c = b*4 + g."""
    scale = 1.0 / math.sqrt(DK)
    wq_s = (wq * scale).astype(np.float32)
    bq_s = (bq * scale).astype(np.float32)
    xT = {}
    for b in range(B):
        xT["q", b] = np.ascontiguousarray(q[b].T)
        xT["k", b] = np.ascontiguousarray(k[b].T)
        xT["v", b] = np.ascontiguousarray(v[b].T)
    in_maps = []
    for c in range(NCORES):
        b, g = divmod(c, HPC)
        hd = slice(g * GH, (g + 1) * GH)
        in_maps.append({
            "qT": xT["q", b],
            "kT": xT["k", b],
            "vT": xT["v", b],
            "wqT": np.ascontiguousarray(wq_s[hd, :].T),
            "wkT": np.ascontiguousarray(wk[hd, :].T),
            "wvT": np.ascontiguousarray(wv[hd, :].T),
            "woT": np.ascontiguousarray(wo[:, hd].T),
            "bq": np.ascontiguousarray(bq_s[hd].reshape(GH, 1)),
            "bk": np.ascontiguousarray(bk[hd].reshape(GH, 1)),
            "bv": np.ascontiguousarray(bv[hd].reshape(1, GH)),
        })
    return in_maps


def _numpy_reference(q, k, v, mask, wq, bq, wk, bk, wv, bv, wo, bo):
    """Exact fp32 fallback (only used if mask has zeros)."""
    qh = (q @ wq.T + bq).reshape(B, S, H, DK).transpose(0, 2, 1, 3)
    kh = (k @ wk.T + bk).reshape(B, S, H, DK).transpose(0, 2, 1, 3)
    vh = (v @ wv.T + bv).reshape(B, S, H, DK).transpose(0, 2, 1, 3)
    out = np.zeros((B, S, D), np.float32)
    for b in range(B):
        for h in range(H):
            sc = (qh[b, h] @ kh[b, h].T) / math.sqrt(DK)
            sc = np.where(mask[0, 0] == 0, np.float32(-1e9), sc)
            sc = sc - sc.max(axis=-1, keepdims=True)
            e = np.exp(sc)
            p = e / e.sum(axis=-1, keepdims=True)
            out[b, :, h * DK:(h + 1) * DK] = p @ vh[b, h]
    return out.reshape(B * S, D) @ wo.T + bo


def get_runner(loop_r=1, parts=('proj', 'attn', 'out')):
    key = ("runner", loop_r, tuple(parts))
    if key not in _STATE:
        nc = _build(loop_r=loop_r, parts=parts)
        _STATE[key] = _Runner(nc, NCORES)
    return _STATE[key]


def kernel(q, k, v, mask, wq, bq, wk, bk, wv, bv, wo, bo):
    q = np.asarray(q, np.float32)
    k = np.asarray(k, np.float32)
    v = np.asarray(v, np.float32)
    mask = np.asarray(mask)
    wq = np.asarray(wq, np.float32); bq = np.asarray(bq, np.float32)
    wk = np.asarray(wk, np.float32); bk = np.asarray(bk, np.float32)
    wv = np.asarray(wv, np.float32); bv = np.asarray(bv, np.float32)
    wo = np.asarray(wo, np.float32); bo = np.asarray(bo, np.float32)

    if np.any(mask == 0):
        out = _numpy_reference(q, k, v, mask, wq, bq, wk, bk, wv, bv, wo, bo)
        return out.reshape(B, S, D).astype(np.float32)

    r = get_runner()
    in_maps = _make_in_maps(q, k, v, wq, bq, wk, bk, wv, bv, wo)
    outs = r.run(r.put_inputs(in_maps))
    res = r.results(outs)
    full = np.zeros((B, S, D), np.float32)
    for c in range(NCORES):
        b = c // HPC
        full[b] += res[c]["out"]
    full += bo[None, None, :]
    return full


# revision 14
# speedup vs baseline: 1.7119x; 1.2356x over previous
"""MultiHeadAttention forward on 8 Trainium2 NeuronCores (Bass/Tile).

Problem: B=2, S=2048, D=1024, H=16 heads (dk=64), fp32, mask all-ones.

Sharding: core c = b*4 + g handles batch b and head group g (4 heads).
Data parallel over B, tensor parallel over heads; w_o row-wise with the
partial-output reduction done host-side (summing 4 fp32 partials).

Device kernel per core (all matmuls in float32r = full-rate fp32):
  1. projections: qhT/khT = (w q)^T layouts [256, 2048] (head dim on
     partitions), vh natural [s, dv] per k-tile, biases fused.
  2. attention per q-chunk of 256: scores k-major [k, q] via row-packed
     K=64 head pairs; exp on ScalarE (PSUM -> SBUF, strided over 4
     half-used banks); PV with stationary [vh | ones] so the softmax
     denominator lands replicated on partitions 64-127 of the ctx bank.
  3. normalize: den -> DMA partition shift -> reciprocal_approx -> TT mul,
     writing the stacked ctx^T tiles used as the output-proj stationary.
  4. output projection -> partial out [2048, 1024] per core.

Host: shards/transposes inputs, runs SPMD over 8 cores, sums group
partials per batch, adds bo.
"""
import math

import numpy as np

B, S, D, H = 2, 2048, 1024, 16
DK = D // H          # 64
HPC = H // 4         # 4 heads per core
NCORES = 8
NT = S // 128        # 16 k-tiles / s-tiles
ND = D // 128        # 8 d-tiles
QC = 256             # q-chunk (f32r moving-operand limit)
NQC = S // QC        # 8
GH = HPC * DK        # 256 output dims per group

_STATE = {}


def _build(loop_r=1, parts=('proj', 'attn', 'out')):
    """Build the Bass program (shared by all 8 cores; inputs differ)."""
    from contextlib import ExitStack

    import concourse.tile as tile
    from concourse import bacc, mybir

    F32 = mybir.dt.float32
    F32R = mybir.dt.float32r
    EXP = mybir.ActivationFunctionType.Exp

    nc = bacc.Bacc("TRN2", target_bir_lowering=False, debug=False,
                   num_devices=NCORES)

    qT_ext = nc.dram_tensor("qT", [D, S], F32R, kind="ExternalInput").ap()
    kT_ext = nc.dram_tensor("kT", [D, S], F32R, kind="ExternalInput").ap()
    vT_ext = nc.dram_tensor("vT", [D, S], F32R, kind="ExternalInput").ap()
    wqT_ext = nc.dram_tensor("wqT", [D, GH], F32R, kind="ExternalInput").ap()
    wkT_ext = nc.dram_tensor("wkT", [D, GH], F32R, kind="ExternalInput").ap()
    wvT_ext = nc.dram_tensor("wvT", [D, GH], F32R, kind="ExternalInput").ap()
    woT_ext = nc.dram_tensor("woT", [GH, D], F32R, kind="ExternalInput").ap()
    bq_ext = nc.dram_tensor("bq", [GH, 1], F32, kind="ExternalInput").ap()
    bk_ext = nc.dram_tensor("bk", [GH, 1], F32, kind="ExternalInput").ap()
    bv_ext = nc.dram_tensor("bv", [1, GH], F32R, kind="ExternalInput").ap()
    out_ext = nc.dram_tensor("out", [S, D], F32, kind="ExternalOutput").ap()

    with tile.TileContext(nc) as tc, ExitStack() as ctx:
        # persistent pools
        cst = ctx.enter_context(tc.tile_pool(name="cst", bufs=1))
        wp = ctx.enter_context(tc.tile_pool(name="wp", bufs=1))
        actp = ctx.enter_context(tc.tile_pool(name="actp", bufs=1))
        xs = ctx.enter_context(tc.tile_pool(name="xs", bufs=2))
        pp = ctx.enter_context(tc.tile_pool(name="pp", bufs=3))
        sm = ctx.enter_context(tc.tile_pool(name="sm", bufs=1))
        ob = ctx.enter_context(tc.tile_pool(name="ob", bufs=2))
        ps = ctx.enter_context(tc.tile_pool(name="ps", bufs=2, space="PSUM"))

        def body():
            # ---- constants / weights ----
            ones_f = cst.tile([128, 128], F32, tag="ones_f")
            nc.vector.memset(ones_f[:], 1.0)
            ones_r = cst.tile([128, 128], F32R, tag="ones_r")
            nc.vector.tensor_copy(ones_r[:], ones_f[:])

            bq_sb = cst.tile([128, 2], F32, tag="bq_sb")
            bk_sb = cst.tile([128, 2], F32, tag="bk_sb")
            for i in range(2):
                nc.sync.dma_start(bq_sb[:, i:i + 1], bq_ext[i * 128:(i + 1) * 128, :])
                nc.sync.dma_start(bk_sb[:, i:i + 1], bk_ext[i * 128:(i + 1) * 128, :])
            bv_sb = cst.tile([1, GH], F32R, tag="bv_sb")
            nc.sync.dma_start(bv_sb[:], bv_ext[:])

            wq_sb = wp.tile([128, ND * GH], F32R, tag="wq_sb")
            wk_sb = wp.tile([128, ND * GH], F32R, tag="wk_sb")
            wv_sb = wp.tile([128, ND * GH], F32R, tag="wv_sb")
            for dt_ in range(ND):
                sl = slice(dt_ * GH, (dt_ + 1) * GH)
                nc.sync.dma_start(wq_sb[:, sl], wqT_ext[dt_ * 128:(dt_ + 1) * 128, :])
                nc.sync.dma_start(wk_sb[:, sl], wkT_ext[dt_ * 128:(dt_ + 1) * 128, :])
                nc.sync.dma_start(wv_sb[:, sl], wvT_ext[dt_ * 128:(dt_ + 1) * 128, :])
            wo_sb = wp.tile([128, 2 * D], F32R, tag="wo_sb")
            nc.sync.dma_start(wo_sb[:, 0:D], woT_ext[0:128, :])
            nc.sync.dma_start(wo_sb[:, D:2 * D], woT_ext[128:256, :])

            # ---- projections: q, k -> qhT/khT [2 x [128, S]] ----
            qhT = [actp.tile([128, S], F32R, tag=f"qhT{i}", name=f"qhT{i}") for i in range(2)]
            khT = [actp.tile([128, S], F32R, tag=f"khT{i}", name=f"khT{i}") for i in range(2)]

            for x_ext, w_sb, b_sb, dst in ((qT_ext, wq_sb, bq_sb, qhT),
                                           (kT_ext, wk_sb, bk_sb, khT)):
                xv = x_ext.rearrange("(a p) s -> p a s", p=128)
                for qtr in range(4):
                    hs = slice(qtr * 512, (qtr + 1) * 512)
                    xh = xs.tile([128, ND * 512], F32R, tag="xh")
                    nc.sync.dma_start(
                        xh[:].rearrange("p (a s) -> p a s", a=ND),
                        xv[:, :, hs])
                    for sc in range(2):
                        for i in range(2):
                            acc = ps.tile([128, QC], F32,
                                          tag=("st" if (sc * 2 + i) % 2 == 0
                                               else "ctx"))
                            for dt_ in range(ND):
                                nc.tensor.matmul(
                                    acc[:],
                                    w_sb[:, dt_ * GH + i * 128:
                                         dt_ * GH + (i + 1) * 128],
                                    xh[:, dt_ * 512 + sc * QC:
                                       dt_ * 512 + (sc + 1) * QC],
                                    start=(dt_ == 0), stop=(dt_ == ND - 1))
                            nc.vector.tensor_scalar_add(
                                dst[i][:, qtr * 512 + sc * QC:
                                       qtr * 512 + (sc + 1) * QC],
                                acc[:], b_sb[:, i:i + 1])

            # ---- projection: v -> vh_aug tiles [128, 512] per k-tile ----
            # head h at cols h*128 : [vh 64 | ones 64]
            vh = [actp.tile([128, 4 * 128], F32R, tag=f"vh{t}", name=f"vh{t}")
                  for t in range(NT)]
            for t in range(NT):
                dst4 = vh[t][:].rearrange("p (h c) -> p h c", h=4)
                nc.vector.tensor_copy(
                    dst4[:, :, 64:128],
                    ones_r[:, 0:64].unsqueeze(1).broadcast_to((128, 4, 64)))
            vv = vT_ext.rearrange("(a p) s -> p a s", p=128)
            for qtr in range(4):
                hs = slice(qtr * 512, (qtr + 1) * 512)
                vht = xs.tile([128, ND * 512], F32R, tag="xh")
                nc.sync.dma_start(
                    vht[:].rearrange("p (a s) -> p a s", a=ND),
                    vv[:, :, hs])
                for st8 in range(4):
                    t = qtr * 4 + st8
                    acc = ps.tile([128, GH], F32,
                                  tag=("st" if st8 % 2 == 0 else "ctx"))
                    for dt_ in range(ND):
                        nc.tensor.matmul(
                            acc[:],
                            vht[:, dt_ * 512 + st8 * 128:
                                dt_ * 512 + (st8 + 1) * 128],
                            wv_sb[:, dt_ * GH:(dt_ + 1) * GH],
                            start=(dt_ == 0), stop=False)
                    nc.tensor.matmul(acc[:], ones_r[0:1, 0:128], bv_sb[:],
                                     start=False, stop=True)
                    nc.vector.tensor_copy(
                        vh[t][:].rearrange("p (h c) -> p h c", h=4)[:, :, 0:64],
                        acc[:].rearrange("p (h c) -> p h c", h=4))

            if 'attn' not in parts:
                # drain: touch outputs so they're written
                o_sb0 = ob.tile([128, D], F32, tag="o_sb")
                nc.vector.tensor_copy(o_sb0[:, 0:S // NT], qhT[0][:, 0:S // NT])
                nc.sync.dma_start(out_ext[0:128, :], o_sb0[:])
                return
            # ---- attention ----
            # stacked normalized ctx^T per pair: [128, S] (A rows 0-63 etc.)
            ctxT = [actp.tile([128, S], F32R, tag=f"ctxT{pr}", name=f"ctxT{pr}")
                    for pr in range(2)]

            # Two passes over head pairs; st/ctx double-buffered so the
            # PE->ACT->PE chain never stalls.
            def scores(pr, t, qsl, slot):
                for hh in range(2):
                    nc.tensor.matmul(
                        slot[:, hh * 512:hh * 512 + QC],
                        khT[pr][hh * 64:(hh + 1) * 64, t * 128:(t + 1) * 128],
                        qhT[pr][hh * 64:(hh + 1) * 64, qsl],
                        start=True, stop=True)

            for pr in range(2):
                for qc in range(NQC):
                    qsl = slice(qc * QC, (qc + 1) * QC)
                    ctx_ps = ps.tile([128, 1024], F32, tag="ctx",
                                     name="ctx_ps")
                    slots = {}
                    slots[0] = ps.tile([128, 1024], F32, tag="st", name="st")
                    scores(pr, 0, qsl, slots[0])
                    for t in range(NT):
                        p_sb = pp.tile([128, 512], F32R, tag="p_sb",
                                       name="p_sb")
                        if 'noexp' in parts:
                            nc.vector.tensor_copy(
                                p_sb[:].rearrange("p (h c) -> p h c", h=2),
                                slots[t][:].rearrange("p (h c) -> p h c",
                                                      h=2)[:, :, 0:QC])
                        else:
                            nc.scalar.activation(
                                p_sb[:].rearrange("p (h c) -> p h c", h=2),
                                slots[t][:].rearrange("p (h c) -> p h c",
                                                      h=2)[:, :, 0:QC],
                                EXP)
                        if t + 1 < NT:
                            slots[t + 1] = ps.tile([128, 1024], F32,
                                                   tag="st", name="st")
                            scores(pr, t + 1, qsl, slots[t + 1])
                            del slots[t]
                        for hh in range(2):
                            h = pr * 2 + hh
                            nc.tensor.matmul(
                                ctx_ps[:, hh * 512:hh * 512 + QC],
                                vh[(t if 'nopv' not in parts else 0)]
                                [:, h * 128:(h + 1) * 128],
                                p_sb[:, hh * QC:(hh + 1) * QC],
                                start=(t == 0), stop=(t == NT - 1))

                    if 'nonorm' in parts:
                        nc.vector.tensor_copy(
                            ctxT[pr][:, qsl].rearrange("p (h c) -> p h c", h=1),
                            ctx_ps[:, 0:QC].rearrange("p (h c) -> p h c", h=1))
                        continue
                    # normalize: den rows 64-127 -> shift -> recip -> TT
                    den_sb = sm.tile([128, 512], F32, tag="den_sb")
                    nc.vector.tensor_copy(
                        den_sb[64:128, :].rearrange("p (h c) -> p h c", h=2),
                        ctx_ps[:].rearrange("p (h c) -> p h c",
                                            h=2)[64:128, :, 0:QC])
                    den_lo = sm.tile([128, 512], F32, tag="den_lo")
                    nc.scalar.dma_start(den_lo[0:64, :], den_sb[64:128, :])
                    recip = sm.tile([128, 512], F32, tag="recip")
                    nc.vector.reciprocal_approx_fast(recip[0:64, :],
                                                     den_lo[0:64, :])
                    bd = sm.tile([128, 256], F32R, tag="bd")
                    nc.vector.tensor_mul(
                        ctxT[pr][0:64, qsl], ctx_ps[0:64, 0:QC],
                        recip[0:64, 0:QC])
                    nc.vector.tensor_mul(
                        bd[0:64, :], ctx_ps[0:64, 512:512 + QC],
                        recip[0:64, QC:2 * QC])
                    nc.scalar.dma_start(ctxT[pr][64:128, qsl], bd[0:64, :])

            # ---- output projection ----
            for s_t in range(NT):
                o_sb = ob.tile([128, D], F32, tag="o_sb")
                for nh in range(2):
                    op = ps.tile([128, 1024], F32,
                                 tag=("st" if nh == 0 else "ctx"),
                                 name=f"op{nh}")
                    for n2 in range(2):
                        n = nh * 2 + n2
                        for pr in range(2):
                            nc.tensor.matmul(
                                op[:, n2 * 512:n2 * 512 + QC],
                                ctxT[pr][:, s_t * 128:(s_t + 1) * 128],
                                wo_sb[:, pr * D + n * QC:pr * D + (n + 1) * QC],
                                start=(pr == 0), stop=(pr == 1))
                    nc.vector.tensor_copy(
                        o_sb[:, nh * 512:(nh + 1) * 512]
                        .rearrange("p (n c) -> p n c", n=2),
                        op[:].rearrange("p (n c) -> p n c", n=2)[:, :, 0:QC])
                nc.sync.dma_start(out_ext[s_t * 128:(s_t + 1) * 128, :],
                                  o_sb[:])

        if loop_r > 1:
            with tc.For_i(0, loop_r, 1):
                body()
        else:
            body()

    nc.compile()
    return nc


class _Runner:
    """SPMD runner on 8 cores via the axon PJRT path (no re-trace)."""

    def __init__(self, nc, n_cores):
        import jax
        from jax.sharding import Mesh, PartitionSpec
        from jax.experimental.shard_map import shard_map
        import concourse.mybir as mybir
        from concourse import bass2jax

        bass2jax.install_neuronx_cc_hook()
        self._jax = jax
        pname = nc.partition_id_tensor.name if nc.partition_id_tensor else None
        in_names, out_names, out_avals, zero_outs = [], [], [], []
        for alloc in nc.m.functions[0].allocations:
            if not isinstance(alloc, mybir.MemoryLocationSet):
                continue
            name = alloc.memorylocations[0].name
            if alloc.kind == "ExternalInput":
                if name != pname:
                    in_names.append(name)
            elif alloc.kind == "ExternalOutput":
                shape = tuple(alloc.tensor_shape)
                dtype = mybir.dt.np(alloc.dtype)
                out_names.append(name)
                out_avals.append(jax.core.ShapedArray(shape, dtype))
                zero_outs.append(np.zeros(shape, dtype))
        self.in_names, self.out_names = in_names, out_names
        self.out_avals, self.zero_outs = out_avals, zero_outs
        self.n_cores = n_cores
        all_in = list(in_names) + list(out_names) + ([pname] if pname else [])

        def _body(*args):
            operands = list(args)
            if pname is not None:
                operands.append(bass2jax.partition_id_tensor())
            return tuple(bass2jax._bass_exec_p.bind(
                *operands, out_avals=tuple(out_avals), in_names=tuple(all_in),
                out_names=tuple(out_names), lowering_input_output_aliases=(),
                sim_require_finite=True, sim_require_nnan=True, nc=nc))

        devices = jax.devices()[:n_cores]
        assert len(devices) >= 1
        self.mesh = Mesh(np.asarray(devices), ("core",))
        spec = PartitionSpec("core")
        n_args = len(in_names) + len(out_names)
        self.fn = jax.jit(
            shard_map(_body, mesh=self.mesh, in_specs=(spec,) * n_args,
                      out_specs=(spec,) * len(out_names), check_rep=False),
            keep_unused=True)
        self.sharding = jax.sharding.NamedSharding(self.mesh, spec)

    def put_inputs(self, in_maps):
        jax = self._jax
        args = []
        for name in self.in_names:
            cat = np.concatenate([np.ascontiguousarray(m[name])
                                  for m in in_maps], axis=0)
            args.append(jax.device_put(cat, self.sharding))
        for z in self.zero_outs:
            cat = np.zeros((self.n_cores * z.shape[0], *z.shape[1:]), z.dtype)
            args.append(jax.device_put(cat, self.sharding))
        return args

    def run(self, args):
        outs = self.fn(*args)
        self._jax.block_until_ready(outs)
        return outs

    def results(self, outs):
        res = []
        for c in range(self.n_cores):
            d = {}
            for i, name in enumerate(self.out_names):
                d[name] = np.asarray(outs[i]).reshape(
                    self.n_cores, *self.out_avals[i].shape)[c]
            res.append(d)
        return res


def _make_in_maps(q, k, v, wq, bq, wk, bk, wv, bv, wo):
    """Host-side sharding/layout prep. Core c = b*4 + g."""
    scale = 1.0 / math.sqrt(DK)
    wq_s = (wq * scale).astype(np.float32)
    bq_s = (bq * scale).astype(np.float32)
    xT = {}
    for b in range(B):
        xT["q", b] = np.ascontiguousarray(q[b].T)
        xT["k", b] = np.ascontiguousarray(k[b].T)
        xT["v", b] = np.ascontiguousarray(v[b].T)
    in_maps = []
    for c in range(NCORES):
        b, g = divmod(c, HPC)
        hd = slice(g * GH, (g + 1) * GH)
        in_maps.append({
            "qT": xT["q", b],
            "kT": xT["k", b],
            "vT": xT["v", b],
            "wqT": np.ascontiguousarray(wq_s[hd, :].T),
            "wkT": np.ascontiguousarray(wk[hd, :].T),
            "wvT": np.ascontiguousarray(wv[hd, :].T),
            "woT": np.ascontiguousarray(wo[:, hd].T),
            "bq": np.ascontiguousarray(bq_s[hd].reshape(GH, 1)),
            "bk": np.ascontiguousarray(bk[hd].reshape(GH, 1)),
            "bv": np.ascontiguousarray(bv[hd].reshape(1, GH)),
        })
    return in_maps


def _numpy_reference(q, k, v, mask, wq, bq, wk, bk, wv, bv, wo, bo):
    """Exact fp32 fallback (only used if mask has zeros)."""
    qh = (q @ wq.T + bq).reshape(B, S, H, DK).transpose(0, 2, 1, 3)
    kh = (k @ wk.T + bk).reshape(B, S, H, DK).transpose(0, 2, 1, 3)
    vh = (v @ wv.T + bv).reshape(B, S, H, DK).transpose(0, 2, 1, 3)
    out = np.zeros((B, S, D), np.float32)
    for b in range(B):
        for h in range(H):
            sc = (qh[b, h] @ kh[b, h].T) / math.sqrt(DK)
            sc = np.where(mask[0, 0] == 0, np.float32(-1e9), sc)
            sc = sc - sc.max(axis=-1, keepdims=True)
            e = np.exp(sc)
            p = e / e.sum(axis=-1, keepdims=True)
            out[b, :, h * DK:(h + 1) * DK] = p @ vh[b, h]
    return out.reshape(B * S, D) @ wo.T + bo


def get_runner(loop_r=1, parts=('proj', 'attn', 'out')):
    key = ("runner", loop_r, tuple(parts))
    if key not in _STATE:
        nc = _build(loop_r=loop_r, parts=parts)
        _STATE[key] = _Runner(nc, NCORES)
    return _STATE[key]


def kernel(q, k, v, mask, wq, bq, wk, bk, wv, bv, wo, bo):
    q = np.asarray(q, np.float32)
    k = np.asarray(k, np.float32)
    v = np.asarray(v, np.float32)
    mask = np.asarray(mask)
    wq = np.asarray(wq, np.float32); bq = np.asarray(bq, np.float32)
    wk = np.asarray(wk, np.float32); bk = np.asarray(bk, np.float32)
    wv = np.asarray(wv, np.float32); bv = np.asarray(bv, np.float32)
    wo = np.asarray(wo, np.float32); bo = np.asarray(bo, np.float32)

    if np.any(mask == 0):
        out = _numpy_reference(q, k, v, mask, wq, bq, wk, bk, wv, bv, wo, bo)
        return out.reshape(B, S, D).astype(np.float32)

    r = get_runner()
    in_maps = _make_in_maps(q, k, v, wq, bq, wk, bk, wv, bv, wo)
    outs = r.run(r.put_inputs(in_maps))
    res = r.results(outs)
    full = np.zeros((B, S, D), np.float32)
    for c in range(NCORES):
        b = c // HPC
        full[b] += res[c]["out"]
    full += bo[None, None, :]
    return full
